# revision 3
# baseline (speedup 1.0000x reference)
"""GATv2 (3 layers, self-loops, segment softmax) on 8 Trainium2 NeuronCores.

Strategy (per spec sharding hint): nodes sharded contiguously across 8 cores;
edges routed to the core owning their dst; per core, edges sorted by dst and
grouped into 128-dst blocks x 128-edge chunks; per layer each core computes
xl/xr for its shard (PE), AllGathers the xl table, then runs the edge phase:
indirect-DMA gathers of xl[src]/xr[dst], score computation on DVE/ACT, and a
one-hot matmul (PE) that does the per-block segment reduction of both the
softmax numerator and denominator in one PSUM accumulation.

Self-contained: hardcodes problem shapes; no sibling imports.
"""
import numpy as np

P = 128          # partitions / block size / chunk size
SC = 4           # chunks per superchunk (batched gathers + elementwise)
NEG_SLOPE = 0.2


# ---------------------------------------------------------------- host prep

def prep_edges(src, dst, N, ncores):
    """Route edges to dst-owning cores, sort by dst, pack into block/chunk slots.

    Returns (Nshard, nblk, NSC, ids) where ids is int32
    [ncores, nblk, NSC, P, 3*SC]: cols [0:SC] global src id, [SC:2*SC] local
    dst id, [2*SC:3*SC] float32-bitcast block-relative dst (1e6 sentinel pad).
    """
    Nshard = ((N + ncores * P - 1) // (ncores * P)) * P
    nblk = Nshard // P
    core = dst // Nshard
    percore = []
    maxch = 1
    for c in range(ncores):
        m = core == c
        s = src[m]
        dl = (dst[m] - c * Nshard).astype(np.int64)
        o = np.argsort(dl, kind='stable')
        s, dl = s[o], dl[o]
        blk = dl // P
        counts = np.bincount(blk, minlength=nblk)
        maxch = max(maxch, int(np.max((counts + P - 1) // P)))
        percore.append((s, dl, blk, counts))
    NSC = (maxch + SC - 1) // SC
    CH = NSC * SC
    ids = np.zeros((ncores, nblk, CH, P, 3), np.int32)
    sentinel = np.float32(1e6).view(np.int32)
    ids[:, :, :, :, 2] = sentinel
    for c in range(ncores):
        s, dl, blk, counts = percore[c]
        starts = np.zeros(nblk, np.int64)
        starts[1:] = np.cumsum(counts)[:-1]
        pos = np.arange(len(s)) - starts[blk]        # rank within block
        ch = pos // P
        p = pos % P
        ids[c, blk, ch, p, 0] = s
        ids[c, blk, ch, p, 1] = dl
        ids[c, blk, ch, p, 2] = (dl - blk * P).astype(np.float32).view(np.int32)
    # [nblk, CH, P, 3] -> [nblk, NSC, P, 3*SC] with cols grouped (src*SC, dst*SC, rel*SC)
    ids = ids.reshape(ncores, nblk, NSC, SC, P, 3)
    ids = ids.transpose(0, 1, 2, 4, 5, 3)            # [c, nblk, NSC, P, 3, SC]
    ids = ids.reshape(ncores, nblk, NSC, P, 3 * SC)
    return Nshard, nblk, NSC, np.ascontiguousarray(ids)


# ---------------------------------------------------------------- bass build

def build_program(ncores, Nshard, nblk, NSC, dims_in, H, C, use_collective=True):
    import concourse.bass as bass
    import concourse.mybir as mybir
    from concourse import bacc
    from concourse.tile import TileContext

    D = H * C
    W = D + H
    L = len(dims_in)
    CH = NSC * SC
    Np = Nshard * ncores
    f32, i32 = mybir.dt.float32, mybir.dt.int32
    AF = mybir.ActivationFunctionType
    OP = mybir.AluOpType

    nc = bacc.Bacc()
    x0 = nc.declare_dram_parameter("x0", [Nshard, dims_in[0]], f32, isOutput=False)
    ids = nc.declare_dram_parameter("ids", [nblk, NSC, P, 3 * SC], i32, isOutput=False)
    wparams = []
    for l in range(L):
        din = dims_in[l]
        wparams.append((
            nc.declare_dram_parameter(f"Wl{l}", [din, D], f32, isOutput=False),
            nc.declare_dram_parameter(f"Wr{l}", [din, D], f32, isOutput=False),
            nc.declare_dram_parameter(f"attb{l}", [P, D], f32, isOutput=False),
            nc.declare_dram_parameter(f"biasb{l}", [P, D], f32, isOutput=False),
        ))
    ident_in = nc.declare_dram_parameter("ident", [P, P], f32, isOutput=False)
    iota_in = nc.declare_dram_parameter("iota", [P, P], f32, isOutput=False)
    y = nc.declare_dram_parameter("y", [Nshard, D], f32, isOutput=True)

    xl_sh = nc.dram_tensor("xl_sh", [Nshard, D], f32)
    xl_full = nc.dram_tensor("xl_full", [Np, D], f32, addr_space="Shared")
    xr_tab = nc.dram_tensor("xr_tab", [Nshard, D], f32)
    xmid = [nc.dram_tensor(f"xmid{i}", [Nshard, D], f32) for i in range(L - 1)]

    with TileContext(nc) as tc:
        with (
            tc.tile_pool(name="const", bufs=1) as cp,
            tc.tile_pool(name="ab", bufs=3) as abp,
            tc.tile_pool(name="abps", bufs=2, space="PSUM") as abps,
            tc.tile_pool(name="edge", bufs=3) as ep,
            tc.tile_pool(name="blkps", bufs=2, space="PSUM") as blkps,
            tc.tile_pool(name="fin", bufs=2) as fp,
        ):
            ident = cp.tile([P, P], f32, tag="ident")
            nc.sync.dma_start(out=ident[:], in_=ident_in[:])
            iota = cp.tile([P, P], f32, tag="iota")
            nc.sync.dma_start(out=iota[:], in_=iota_in[:])
            wts = []
            for l in range(L):
                din = dims_in[l]
                wl = cp.tile([din, D], f32, tag=f"wl{l}")
                nc.sync.dma_start(out=wl[:], in_=wparams[l][0][:])
                wr = cp.tile([din, D], f32, tag=f"wr{l}")
                nc.sync.dma_start(out=wr[:], in_=wparams[l][1][:])
                attb = cp.tile([P, D], f32, tag=f"attb{l}")
                nc.sync.dma_start(out=attb[:], in_=wparams[l][2][:])
                biasb = cp.tile([P, D], f32, tag=f"biasb{l}")
                nc.sync.dma_start(out=biasb[:], in_=wparams[l][3][:])
                wts.append((wl, wr, attb, biasb))

            for l in range(L):
                din = dims_in[l]
                x_cur = x0 if l == 0 else xmid[l - 1]
                x_out = y if l == L - 1 else xmid[l]
                wl, wr, attb, biasb = wts[l]

                # ---- node transform: xl_sh = x @ Wl, xr_tab = x @ Wr
                for t in range(nblk):
                    xc = abp.tile([P, din], f32, tag="xc")
                    nc.sync.dma_start(out=xc[:], in_=x_cur[t * P:(t + 1) * P, :])
                    xt_ps = abps.tile([din, P], f32, tag="xtps")
                    nc.tensor.transpose(out=xt_ps[:], in_=xc[:], identity=ident[:])
                    xt = abp.tile([din, P], f32, tag="xt")
                    nc.scalar.activation(out=xt[:], in_=xt_ps[:], func=AF.Copy)
                    mml = abps.tile([P, D], f32, tag="mml")
                    nc.tensor.matmul(out=mml[:], lhsT=xt[:], rhs=wl[:], start=True, stop=True)
                    mmr = abps.tile([P, D], f32, tag="mmr")
                    nc.tensor.matmul(out=mmr[:], lhsT=xt[:], rhs=wr[:], start=True, stop=True)
                    sxl = abp.tile([P, D], f32, tag="sxl")
                    nc.vector.tensor_copy(out=sxl[:], in_=mml[:])
                    nc.sync.dma_start(out=xl_sh[t * P:(t + 1) * P, :], in_=sxl[:])
                    sxr = abp.tile([P, D], f32, tag="sxr")
                    nc.scalar.activation(out=sxr[:], in_=mmr[:], func=AF.Copy)
                    nc.sync.dma_start(out=xr_tab[t * P:(t + 1) * P, :], in_=sxr[:])

                # ---- gather table for xl across all cores
                if use_collective:
                    nc.gpsimd.collective_compute(
                        "AllGather", OP.bypass,
                        replica_groups=[list(range(ncores))],
                        ins=[xl_sh[:]], outs=[xl_full[:]],
                    )
                else:
                    nc.sync.dma_start(out=xl_full[:], in_=xl_sh[:])

                # ---- edge phase
                for blk in range(nblk):
                    ps = blkps.tile([P, W], f32, tag="ps")
                    for sc in range(NSC):
                        idst = ep.tile([P, 3 * SC], i32, tag="idst")
                        nc.sync.dma_start(out=idst[:], in_=ids[blk, sc, :, :])
                        xl_s = ep.tile([P, SC * D], f32, tag="xls")
                        xr_d = ep.tile([P, SC * D], f32, tag="xrd")
                        for k in range(SC):
                            # HW indirect DMA: one gathered row per partition per call
                            nc.gpsimd.indirect_dma_start(
                                out=xl_s[:, k * D:(k + 1) * D], out_offset=None, in_=xl_full[:],
                                in_offset=bass.IndirectOffsetOnAxis(ap=idst[:, k:k + 1], axis=0))
                            nc.gpsimd.indirect_dma_start(
                                out=xr_d[:, k * D:(k + 1) * D], out_offset=None, in_=xr_tab[:],
                                in_offset=bass.IndirectOffsetOnAxis(ap=idst[:, SC + k:SC + k + 1], axis=0))
                        g = ep.tile([P, SC * D], f32, tag="g")
                        nc.vector.tensor_tensor(out=g[:], in0=xl_s[:], in1=xr_d[:], op=OP.add)
                        gl = ep.tile([P, SC * D], f32, tag="gl")
                        # leaky_relu(g) = max(0.2*g, g) in one fused DVE op
                        nc.vector.scalar_tensor_tensor(
                            out=gl[:], in0=g[:], scalar=NEG_SLOPE, in1=g[:],
                            op0=OP.mult, op1=OP.max)
                        ge = ep.tile([P, SC * D], f32, tag="ge")
                        nc.vector.tensor_tensor(
                            out=ge[:].rearrange("p (s d) -> p s d", s=SC),
                            in0=gl[:].rearrange("p (s d) -> p s d", s=SC),
                            in1=attb[:].unsqueeze(1).to_broadcast([P, SC, D]),
                            op=OP.mult)
                        e = ep.tile([P, SC * H], f32, tag="e")
                        nc.vector.tensor_reduce(
                            out=e[:].rearrange("p (s h) -> p s h", s=SC),
                            in_=ge[:].rearrange("p (s h c) -> p s h c", s=SC, h=H),
                            axis=mybir.AxisListType.X, op=OP.add)
                        vals = ep.tile([P, SC * W], f32, tag="vals")
                        vals_v = vals[:].rearrange("p (s w) -> p s w", s=SC)
                        nc.scalar.activation(
                            out=vals_v[:, :, D:W],
                            in_=e[:].rearrange("p (s h) -> p s h", s=SC),
                            func=AF.Exp)
                        nc.vector.tensor_tensor(
                            out=vals_v[:, :, 0:D].rearrange("p s (h c) -> p s h c", h=H),
                            in0=xl_s[:].rearrange("p (s h c) -> p s h c", s=SC, h=H),
                            in1=vals_v[:, :, D:W].unsqueeze(3).to_broadcast([P, SC, H, C]),
                            op=OP.mult)
                        ot = ep.tile([P, SC * P], f32, tag="ot")
                        nc.vector.tensor_tensor(
                            out=ot[:].rearrange("p (s q) -> p s q", s=SC),
                            in0=idst[:, 2 * SC:3 * SC].bitcast(f32).unsqueeze(2).to_broadcast([P, SC, P]),
                            in1=iota[:].unsqueeze(1).to_broadcast([P, SC, P]),
                            op=OP.is_equal)
                        for k in range(SC):
                            ch = sc * SC + k
                            nc.tensor.matmul(
                                out=ps[:], lhsT=ot[:, k * P:(k + 1) * P],
                                rhs=vals[:, k * W:(k + 1) * W],
                                start=(ch == 0), stop=(ch == CH - 1))
                    # ---- finalize block: div by denom, +bias, elu, store
                    den = fp.tile([P, H], f32, tag="den")
                    nc.vector.tensor_scalar(
                        out=den[:], in0=ps[:, D:W], scalar1=1e-30, scalar2=None, op0=OP.max)
                    r = fp.tile([P, H], f32, tag="r")
                    nc.vector.reciprocal(out=r[:], in_=den[:])
                    o = fp.tile([P, D], f32, tag="o")
                    nc.vector.tensor_tensor(
                        out=o[:].rearrange("p (h c) -> p h c", h=H),
                        in0=ps[:, 0:D].rearrange("p (h c) -> p h c", h=H),
                        in1=r[:].unsqueeze(2).to_broadcast([P, H, C]),
                        op=OP.mult)
                    nc.vector.tensor_tensor(out=o[:], in0=o[:], in1=biasb[:], op=OP.add)
                    t1 = fp.tile([P, D], f32, tag="t1")
                    nc.vector.tensor_scalar(
                        out=t1[:], in0=o[:], scalar1=0.0, scalar2=None, op0=OP.min)
                    nc.scalar.activation(out=t1[:], in_=t1[:], func=AF.Exp)
                    nc.vector.tensor_scalar(
                        out=t1[:], in0=t1[:], scalar1=-1.0, scalar2=None, op0=OP.add)
                    nc.vector.tensor_tensor(out=o[:], in0=o[:], in1=t1[:], op=OP.max)
                    nc.sync.dma_start(out=x_out[blk * P:(blk + 1) * P, :], in_=o[:])
    nc.compile()
    return nc


# ---------------------------------------------------------------- entry

def make_inmaps(inputs, ncores):
    x = np.asarray(inputs['x'], np.float32)
    ei = np.asarray(inputs['edge_index'], np.int32)
    N, F = x.shape
    H, C = np.asarray(inputs['att0']).shape
    D = H * C
    L = 3
    loops = np.arange(N, dtype=np.int32)
    src = np.concatenate([ei[0], loops])
    dst = np.concatenate([ei[1], loops])
    Nshard, nblk, NSC, ids = prep_edges(src, dst, N, ncores)
    xp = np.zeros((Nshard * ncores, F), np.float32)
    xp[:N] = x
    iota = np.broadcast_to(np.arange(P, dtype=np.float32), (P, P)).copy()
    ident = np.eye(P, dtype=np.float32)
    dims_in = [F] + [D] * (L - 1)
    base = {"ident": ident, "iota": iota}
    for l in range(L):
        base[f"Wl{l}"] = np.ascontiguousarray(np.asarray(inputs[f'Wl{l}'], np.float32))
        base[f"Wr{l}"] = np.ascontiguousarray(np.asarray(inputs[f'Wr{l}'], np.float32))
        att = np.asarray(inputs[f'att{l}'], np.float32).reshape(1, D)
        base[f"attb{l}"] = np.broadcast_to(att, (P, D)).copy()
        b = np.asarray(inputs[f'b{l}'], np.float32).reshape(1, D)
        base[f"biasb{l}"] = np.broadcast_to(b, (P, D)).copy()
    in_maps = []
    for c in range(ncores):
        m = dict(base)
        m["x0"] = np.ascontiguousarray(xp[c * Nshard:(c + 1) * Nshard])
        m["ids"] = np.ascontiguousarray(ids[c])
        in_maps.append(m)
    return in_maps, Nshard, nblk, NSC, dims_in, H, C, N, D


def kernel(**inputs):
    from concourse.bass_utils import run_bass_kernel_spmd
    ncores = 8
    in_maps, Nshard, nblk, NSC, dims_in, H, C, N, D = make_inmaps(inputs, ncores)
    nc = build_program(ncores, Nshard, nblk, NSC, dims_in, H, C, use_collective=True)
    res = run_bass_kernel_spmd(nc, in_maps, list(range(ncores)))
    out = np.concatenate([res.results[c]["y"] for c in range(ncores)], axis=0)
    return out[:N].astype(np.float32)


if __name__ == "__main__":
    pass
